# revision 28
# baseline (speedup 1.0000x reference)
"""MoE FFN (SwiGLU, top-2 routing) on 8 Trainium2 NeuronCores.

Strategy (expert-parallel + 4-way intra-expert token split):
  - Host computes the tiny gate (softmax + top-2 + renormalize) in numpy and
    splits each expert's routed tokens into 4 quarters. Experts are grouped
    into two load-balanced quads; core c serves quarter c%4 of each of the 4
    experts in quad c//4. Per-run capacities are the max quarter size over the
    two quads, so all 8 cores run one SPMD program; C=2071 columns/core here,
    vs the 2070 floor implied by the heavier quad — effectively optimal.
  - Each core runs the same Bass/Tile kernel: for each of its 4 runs
    (= expert quarter) Y^T = W2^T @ (silu(W1^T X^T) * (W3^T X^T)), bf16
    matmuls with fp32 PSUM accumulation, activations kept transposed so
    weights are consumed in natural layout as the stationary operand.
  - Host scales each run's output rows by the gate weight and scatter-adds
    into the full (B,T,D) output.

Per-core loop: F in groups of FG f-tiles; phase 1 builds the group's H^T
run-major, phase 2 accumulates Y^T into a resident fp32 SBUF accumulator
(final group sums into a bf16 buffer reusing xt's SBUF and streams out on
the ACT ring). Weights stream through SBUF exactly once per core.

Schedule notes (from perfetto/NTFF traces; mid-run is a single gap-free
tensor-engine slice at ~97% of the bf16 streaming roofline):
  - SDMA engines round-robin ACTIVE queue rings per ~packet, so concurrent
    rings split bandwidth; the startup critical path (strip0 || run-0 x)
    uses the only two genuinely parallel blocks, everything else follows
    FIFO on the SP ring.  Each HWDGE trigger costs ~0.6us of engine time ->
    one descriptor per logical block.
  - Dummy matmuls on zeroed scratch warm the PE HAM clock-gate (1.2->2.4
    GHz after ~3.4us busy) during the initial DMA wait; idle gaps >~3us
    re-throttle it.
  - w2 strips carry dt-pairs (4KB/partition packets) so their ring keeps a
    fair service share against the concurrent yt output stream.
"""

import os
import sys

import numpy as np

for _p in ("/opt/trn_rl_repo", "/root/.axon_site/_ro/trn_rl_repo"):
    if os.path.isdir(_p) and _p not in sys.path:
        sys.path.append(_p)

import ml_dtypes  # noqa: E402
import concourse.bass as bass  # noqa: E402
import concourse.mybir as mybir  # noqa: E402
import concourse.tile as tile  # noqa: E402
from concourse import bacc  # noqa: E402
from concourse.bass_utils import run_bass_kernel_spmd  # noqa: E402

P = 128
TOP_K = 2
N_CORES = 8
SPLIT = 4        # cores (= quarters) per expert
WBUFS = 6        # weight-strip prefetch depth (absorbs startup DMA jitter)

BF16 = mybir.dt.bfloat16
F32 = mybir.dt.float32


def _run_chunks(cap: int, step: int = 512):
    """Split a run of `cap` columns into equal-ish chunks of <= step."""
    n = -(-cap // step)
    base, extra = divmod(cap, n)
    out, c0 = [], 0
    for i in range(n):
        w = base + (1 if i < extra else 0)
        out.append((c0, w))
        c0 += w
    return out


def build_ffn_nc(D: int, F: int, caps: tuple, FG: int = 8) -> bass.Bass:
    """R-run SwiGLU FFN, activations transposed. Run r covers columns
    [off_r, off_r + caps[r]) of xt/yt and uses weight set w1_r/w3_r/w2_r.

    Inputs:  xt (D, C) bf16; per run r: w1_r (D, F), w3_r (D, F),
             w2_r (F, D), all bf16.
    Output:  yt (D, C) bf16, per-run  yt = ((silu(x@w1)*(x@w3)) @ w2)^T.
    """
    R = len(caps)
    C = sum(caps)
    offs = [sum(caps[:r]) for r in range(R)]
    assert D % P == 0 and F % P == 0
    KD, KF = D // P, F // P
    assert KF % FG == 0
    NG = KF // FG
    rchunks = [_run_chunks(cap) for cap in caps]

    nc = bacc.Bacc(None, target_bir_lowering=False)
    xt = nc.dram_tensor("xt", [D, C], BF16, kind="ExternalInput")
    # weights come pre-rearranged strip-major from the host (see
    # _strip_w13/_strip_w2) so every strip load is ONE contiguous descriptor
    # — SP-queue descriptor issue (~0.6us each) is the scarce resource.
    assert KD % 2 == 0
    w13_d, w2_d = [], []
    for r in range(R):
        w13_d.append(nc.dram_tensor(f"w13_{r}", [KF, P, 2, KD, P], BF16,
                                    kind="ExternalInput"))
        # w2 strips carry a dt-PAIR per descriptor: 4KB contiguous per
        # partition doubles this queue's share under the SDMA engines'
        # per-packet round-robin when yt streams out concurrently.
        w2_d.append(nc.dram_tensor(f"w2_{r}", [NG, KD // 2, P, 2, FG, P],
                                   BF16, kind="ExternalInput"))
    # bf16 output halves the store traffic in the DMA-saturated last-group
    # phase 2 (the final sum is rounded once; ~2e-3 extra rel err).
    yt = nc.dram_tensor("yt", [D, C], BF16, kind="ExternalOutput")

    xt_r = xt[:].rearrange("(kd p) c -> p kd c", p=P)
    xt_r_out = yt[:].rearrange("(kd p) c -> p kd c", p=P)

    Silu = mybir.ActivationFunctionType.Silu
    Mult = mybir.AluOpType.mult

    with tile.TileContext(nc) as tc:
        with (
            tc.tile_pool(name="resident", bufs=1) as resident,
            tc.tile_pool(name="wstrips", bufs=3) as wstrips,
            tc.tile_pool(name="tmp", bufs=3) as tmp,
            tc.tile_pool(name="psum", bufs=2, space="PSUM") as psum,
        ):
            xt_sb = resident.tile([P, KD, C], BF16, tag="xt")
            ht = resident.tile([P, FG, C], BF16, tag="ht")
            y_acc = resident.tile([P, KD, C], F32, tag="yacc")

            # Warm the PE clock: HAM un-throttles (1.2 -> 2.4 GHz) only after
            # ~3.4us of sustained matmul activity, so burn the initial DMA
            # wait on dummy matmuls — real MMs then start at full clock.
            warm = resident.tile([P, 512], BF16, tag="warm")
            nc.vector.memset(warm[:], 0.0)
            pswarm = psum.tile([P, 512], F32, tag="pswarm", name="pswarm",
                               bufs=1)
            for _ in range(18):
                nc.tensor.matmul(pswarm, warm[:, :P], warm[:],
                                 start=True, stop=True)
            for _ in range(8):
                # fine-grained tail: keeps the PE busy until the startup x
                # lands without coarse MMs delaying the first real matmul
                nc.tensor.matmul(pswarm[:, :P], warm[:, :P], warm[:, :P],
                                 start=True, stop=True)

            # Startup: the SDMA engines round-robin between ACTIVE queue
            # rings at packet granularity, so concurrent rings DIVIDE
            # bandwidth — and Tile's per-engine scheduling is work-
            # conserving, so a trigger on an idle engine fires immediately
            # no matter where it was emitted. The ONLY way to get strict
            # service order is one ring, emission-ordered: strip0, run-0 x,
            # run-0's remaining strips, then the other runs' x (emitted after
            # run 0's phase-1 strips below), then later strips.
            # Each HWDGE trigger costs ~0.6us of issuing-engine time, so the
            # critical path uses ONE descriptor per logical block.
            # Ring order = [w1 half of strip0][run-0 x][w3 half]: the ps1
            # kd-sweep needs only the w1 half + x, so the first real matmul
            # fires one strip-half (~1us) earlier; the w3 half lands while
            # the ps1 sweep runs. (SWDGE for strip0 measured ~6us first-byte
            # latency — strictly worse; keep everything on the SP ring.)
            # x leads: its completion semaphore (~2.8us write-receipt after
            # the transfer) is the critical gate for the first real matmul,
            # so its transfer starts first; the strip halves land under it.
            pre_strips = {}
            w13s0 = wstrips.tile([P, 2, KD, P], BF16, tag="w13s",
                                 name="w13s0", bufs=WBUFS)
            c0 = caps[0]
            nc.sync.dma_start(xt_sb[:, :, :c0], xt_r[:, :, :c0])
            nc.sync.dma_start(w13s0[:, 0], w13_d[0][0][:, 0])
            nc.sync.dma_start(w13s0[:, 1], w13_d[0][0][:, 1])
            pre_strips[(0, 0)] = w13s0

            def p1_chunk(w13s, ftl, lo, cw):
                ps1 = psum.tile([P, 512], F32, tag="ps1", name="ps1")[:, :cw]
                ps3 = psum.tile([P, 512], F32, tag="ps3", name="ps3")[:, :cw]
                for kd in range(KD):
                    nc.tensor.matmul(
                        ps1, w13s[:, 0, kd, :], xt_sb[:, kd, lo:lo + cw],
                        start=(kd == 0), stop=(kd == KD - 1),
                    )
                for kd in range(KD):
                    nc.tensor.matmul(
                        ps3, w13s[:, 1, kd, :], xt_sb[:, kd, lo:lo + cw],
                        start=(kd == 0), stop=(kd == KD - 1),
                    )
                h1t = tmp.tile([P, 512], BF16, tag="h1t", name="h1t")[:, :cw]
                nc.scalar.activation(h1t, ps1, Silu)
                nc.vector.tensor_tensor(ht[:, ftl, lo:lo + cw], h1t, ps3, op=Mult)

            def get_strip(r, kf):
                if (r, kf) in pre_strips:
                    return pre_strips.pop((r, kf))
                s = wstrips.tile([P, 2, KD, P], BF16, tag="w13s", bufs=WBUFS)
                nc.sync.dma_start(s[:], w13_d[r][kf])
                return s

            for g in range(NG):
                # ---- phase 1: H^T for this f-group, run-major ----
                for r in range(R):
                    off = offs[r]
                    for ftl in range(FG):
                        w13s = get_strip(r, g * FG + ftl)
                        for (cc, cw) in rchunks[r]:
                            p1_chunk(w13s, ftl, off + cc, cw)
                    if g == 0 and r == 0:
                        # runs 1..R-1's x, behind run 0's strips on the SAME
                        # ring: hardware-FIFO ordering keeps the startup
                        # strip stream at full bandwidth; these still land
                        # ~25us before run 1's first matmul needs them.
                        for rr in range(1, R):
                            o = offs[rr]
                            nc.sync.dma_start(
                                xt_sb[:, :, o:o + caps[rr]],
                                xt_r[:, :, o:o + caps[rr]],
                            )
                # ---- phase 2: accumulate Y^T contribution of this group ----
                # Last group: the final sum goes to a bf16 buffer — xt_sb is
                # fully consumed by now, so reuse it (zero extra SBUF) — and
                # yt streams out per dt-row on the otherwise-idle ACT ring,
                # keeping the SP ring free for w2 strips.
                for dtp in range(KD // 2):
                    for r in range(R):
                        off = offs[r]
                        w2s = wstrips.tile([P, 2, FG, P], BF16, tag="w2s",
                                           bufs=WBUFS)
                        nc.sync.dma_start(w2s[:], w2_d[r][g, dtp])
                        for j in range(2):
                            dt = 2 * dtp + j
                            for (cc, cw) in rchunks[r]:
                                lo = off + cc
                                psy = psum.tile(
                                    [P, 512], F32, tag="psy", name="psy", bufs=3
                                )[:, :cw]
                                for ftl in range(FG):
                                    nc.tensor.matmul(
                                        psy, w2s[:, j, ftl, :],
                                        ht[:, ftl, lo:lo + cw],
                                        start=(ftl == 0), stop=(ftl == FG - 1),
                                    )
                                if g == 0:
                                    nc.vector.tensor_copy(
                                        y_acc[:, dt, lo:lo + cw], psy)
                                elif g < NG - 1:
                                    nc.vector.tensor_add(
                                        y_acc[:, dt, lo:lo + cw],
                                        y_acc[:, dt, lo:lo + cw], psy,
                                    )
                                else:
                                    nc.vector.tensor_add(
                                        xt_sb[:, dt, lo:lo + cw],
                                        y_acc[:, dt, lo:lo + cw], psy,
                                    )
                        if g == NG - 1 and dtp == KD // 2 - 1:
                            # per-run drain so the very last descriptor
                            # (issued after the final add) is small
                            for j in range(2):
                                dt = 2 * dtp + j
                                nc.scalar.dma_start(
                                    xt_r_out[:, dt, off:off + caps[r]],
                                    xt_sb[:, dt, off:off + caps[r]],
                                )
                    if g == NG - 1 and dtp < KD // 2 - 1:
                        for j in range(2):
                            dt = 2 * dtp + j
                            nc.scalar.dma_start(
                                xt_r_out[:, dt, :], xt_sb[:, dt, :])
    nc.finalize()
    return nc


_NC_CACHE: dict = {}
last_results = None


def _install_ntff_shim():
    """This container's antenv lacks axon_hooks; recreate the NTFF profile
    hook from trn_boot's ctypes wrapper so trace=True yields profiles."""
    import types
    try:
        import antenv.axon_hooks  # noqa: F401
        return
    except ImportError:
        pass
    try:
        from trn_agent_boot.trn_boot import _ntff_profile_via_ctypes
        hook = _ntff_profile_via_ctypes("/opt/axon/libaxon_pjrt.so")
        mod = types.ModuleType("antenv.axon_hooks")
        mod.get_axon_ntff_profile_hook = lambda: hook
        mod.set_axon_ntff_profile_hook = lambda h: None
        sys.modules["antenv.axon_hooks"] = mod
    except Exception:
        pass


def _get_nc(D, F, caps, FG):
    key = (D, F, tuple(caps), FG)
    if key not in _NC_CACHE:
        _NC_CACHE[key] = build_ffn_nc(D, F, tuple(caps), FG)
    return _NC_CACHE[key]


def _softmax(z):
    e = np.exp(z - z.max(-1, keepdims=True))
    return e / e.sum(-1, keepdims=True)


def _strip_w13(w1, w3, dtype):
    """(D, F) x2 -> (KF, P, 2, KD, P): strip kf holds the w1 and w3 columns
    interleaved as one contiguous 512KB block, laid out exactly as the SBUF
    tile (partition-major, then w1/w3, then kd, then column)."""
    D, F = w1.shape
    KD, KF = D // P, F // P
    a = w1.reshape(KD, P, KF, P).transpose(2, 1, 0, 3)
    b = w3.reshape(KD, P, KF, P).transpose(2, 1, 0, 3)
    return np.ascontiguousarray(np.stack([a, b], axis=2)).astype(dtype)


def _strip_w2(w, FG, dtype):
    """(F, D) -> (NG, KD/2, P, 2, FG, P): strip (g, dtp) holds the dt-pair
    (2*dtp, 2*dtp+1) as one contiguous block, 4KB per partition line."""
    F, D = w.shape
    KD, KF = D // P, F // P
    NG = KF // FG
    return np.ascontiguousarray(
        w.reshape(NG, FG, P, KD // 2, 2, P).transpose(0, 3, 2, 4, 1, 5)
    ).astype(dtype)


def kernel(x, gate_w, w1, w3, w2):
    x = np.asarray(x, dtype=np.float32)
    gate_w = np.asarray(gate_w, dtype=np.float32)
    w1 = np.asarray(w1, dtype=np.float32)
    w3 = np.asarray(w3, dtype=np.float32)
    w2 = np.asarray(w2, dtype=np.float32)

    B, T, D = x.shape
    E, _, F = w1.shape
    N = B * T
    xf = x.reshape(N, D)

    # ---- host gate: softmax + top-2 + renormalize (tiny; replicated) ----
    logits = xf @ gate_w                      # (N, E)
    probs = _softmax(logits)
    top2 = np.argpartition(-probs, TOP_K - 1, axis=-1)[:, :TOP_K]  # (N, 2)
    pw = np.take_along_axis(probs, top2, axis=-1)
    pw = pw / pw.sum(-1, keepdims=True)       # renormalized weights

    # ---- dispatch: gather tokens per expert ----
    tok_ids, tok_wts = [], []
    for e in range(E):
        mask = (top2 == e)
        any_row = mask.any(-1)
        rows = np.nonzero(any_row)[0]
        wts = pw[any_row, :][mask[any_row, :]]
        tok_ids.append(rows)
        tok_wts.append(wts.astype(np.float32))
    counts = np.array([len(r) for r in tok_ids])

    # ---- group experts into two load-balanced quads; run r capacity is the
    # max quarter size over the two quads so one SPMD program fits all cores.
    G = N_CORES // SPLIT                      # number of quads (2)
    order = np.argsort(-counts, kind="stable")
    quads = [order[i::G] for i in range(G)]   # interleaved: balances run caps
    R = len(quads[0])
    caps = [int(-(-max(counts[quads[q][r]] for q in range(G)) // SPLIT))
            for r in range(R)]
    C = sum(caps)
    offs = [sum(caps[:r]) for r in range(R)]

    bf16 = ml_dtypes.bfloat16
    FG = 8
    wq = [(_strip_w13(w1[e], w3[e], bf16),
           _strip_w2(w2[e], FG, bf16)) for e in range(E)]

    nc = _get_nc(D, F, caps, FG)

    in_maps = []
    core_runs = []   # per core: list of (rows, wts, off) per run
    for c in range(N_CORES):
        q, quarter = c // SPLIT, c % SPLIT
        xt_c = np.zeros((D, C), dtype=bf16)
        im = {"xt": xt_c}
        runs = []
        for r in range(R):
            e = int(quads[q][r])
            qs = -(-counts[e] // SPLIT)       # quarter size for this expert
            rows = tok_ids[e][quarter * qs: (quarter + 1) * qs]
            wts = tok_wts[e][quarter * qs: (quarter + 1) * qs]
            xt_c[:, offs[r]: offs[r] + len(rows)] = xf[rows].T.astype(bf16)
            im[f"w13_{r}"], im[f"w2_{r}"] = wq[e]
            runs.append((rows, wts, offs[r]))
        in_maps.append(im)
        core_runs.append(runs)

    trace = os.environ.get("MOE_TRACE", "0") == "1"
    if trace:
        _install_ntff_shim()
    res = run_bass_kernel_spmd(nc, in_maps, list(range(N_CORES)), trace=trace)
    global last_results
    last_results = res

    out = np.zeros((N, D), dtype=np.float32)
    for c in range(N_CORES):
        y = np.asarray(res.results[c]["yt"], dtype=np.float32).T  # (C, D)
        for rows, wts, off in core_runs[c]:
            out[rows] += wts[:, None] * y[off: off + len(rows)]
    return out.reshape(B, T, D)



# revision 29
# speedup vs baseline: 1.0003x; 1.0003x over previous
"""MoE FFN (SwiGLU, top-2 routing) on 8 Trainium2 NeuronCores.

Strategy (expert-parallel + 4-way intra-expert token split):
  - Host computes the tiny gate (softmax + top-2 + renormalize) in numpy and
    splits each expert's routed tokens into 4 quarters. Experts are grouped
    into two load-balanced quads; core c serves quarter c%4 of each of the 4
    experts in quad c//4. Per-run capacities are the max quarter size over the
    two quads, so all 8 cores run one SPMD program; C=2071 columns/core here,
    vs the 2070 floor implied by the heavier quad — effectively optimal.
  - Each core runs the same Bass/Tile kernel: for each of its 4 runs
    (= expert quarter) Y^T = W2^T @ (silu(W1^T X^T) * (W3^T X^T)), bf16
    matmuls with fp32 PSUM accumulation, activations kept transposed so
    weights are consumed in natural layout as the stationary operand.
  - Host scales each run's output rows by the gate weight and scatter-adds
    into the full (B,T,D) output.

Per-core loop: F in groups of FG f-tiles; phase 1 builds the group's H^T
run-major, phase 2 accumulates Y^T into a resident fp32 SBUF accumulator
(final group sums into a bf16 buffer reusing xt's SBUF and streams out on
the ACT ring). Weights stream through SBUF exactly once per core.

Schedule notes (from perfetto/NTFF traces; mid-run is a single gap-free
tensor-engine slice at ~97% of the bf16 streaming roofline):
  - SDMA engines round-robin ACTIVE queue rings per ~packet, so concurrent
    rings split bandwidth; the startup critical path (strip0 || run-0 x)
    uses the only two genuinely parallel blocks, everything else follows
    FIFO on the SP ring.  Each HWDGE trigger costs ~0.6us of engine time ->
    one descriptor per logical block.
  - Dummy matmuls on zeroed scratch warm the PE HAM clock-gate (1.2->2.4
    GHz after ~3.4us busy) during the initial DMA wait; idle gaps >~3us
    re-throttle it.
  - w2 strips carry dt-pairs (4KB/partition packets) so their ring keeps a
    fair service share against the concurrent yt output stream.
"""

import os
import sys

import numpy as np

for _p in ("/opt/trn_rl_repo", "/root/.axon_site/_ro/trn_rl_repo"):
    if os.path.isdir(_p) and _p not in sys.path:
        sys.path.append(_p)

import ml_dtypes  # noqa: E402
import concourse.bass as bass  # noqa: E402
import concourse.mybir as mybir  # noqa: E402
import concourse.tile as tile  # noqa: E402
from concourse import bacc  # noqa: E402
from concourse.bass_utils import run_bass_kernel_spmd  # noqa: E402

P = 128
TOP_K = 2
N_CORES = 8
SPLIT = 4        # cores (= quarters) per expert
WBUFS = 6        # weight-strip prefetch depth (absorbs startup DMA jitter)

BF16 = mybir.dt.bfloat16
F32 = mybir.dt.float32


def _run_chunks(cap: int, step: int = 512):
    """Split a run of `cap` columns into equal-ish chunks of <= step."""
    n = -(-cap // step)
    base, extra = divmod(cap, n)
    out, c0 = [], 0
    for i in range(n):
        w = base + (1 if i < extra else 0)
        out.append((c0, w))
        c0 += w
    return out


def build_ffn_nc(D: int, F: int, caps: tuple, FG: int = 8) -> bass.Bass:
    """R-run SwiGLU FFN, activations transposed. Run r covers columns
    [off_r, off_r + caps[r]) of xt/yt and uses weight set w1_r/w3_r/w2_r.

    Inputs:  xt (D, C) bf16; per run r: w1_r (D, F), w3_r (D, F),
             w2_r (F, D), all bf16.
    Output:  yt (D, C) bf16, per-run  yt = ((silu(x@w1)*(x@w3)) @ w2)^T.
    """
    R = len(caps)
    C = sum(caps)
    offs = [sum(caps[:r]) for r in range(R)]
    assert D % P == 0 and F % P == 0
    KD, KF = D // P, F // P
    assert KF % FG == 0
    NG = KF // FG
    rchunks = [_run_chunks(cap) for cap in caps]

    nc = bacc.Bacc(None, target_bir_lowering=False)
    xt = nc.dram_tensor("xt", [D, C], BF16, kind="ExternalInput")
    # weights come pre-rearranged strip-major from the host (see
    # _strip_w13/_strip_w2) so every strip load is ONE contiguous descriptor
    # — SP-queue descriptor issue (~0.6us each) is the scarce resource.
    assert KD % 2 == 0
    w13_d, w2_d = [], []
    for r in range(R):
        w13_d.append(nc.dram_tensor(f"w13_{r}", [KF, P, 2, KD, P], BF16,
                                    kind="ExternalInput"))
        # w2 strips carry a dt-PAIR per descriptor: 4KB contiguous per
        # partition doubles this queue's share under the SDMA engines'
        # per-packet round-robin when yt streams out concurrently.
        w2_d.append(nc.dram_tensor(f"w2_{r}", [NG, KD // 2, P, 2, FG, P],
                                   BF16, kind="ExternalInput"))
    # bf16 output halves the store traffic in the DMA-saturated last-group
    # phase 2 (the final sum is rounded once; ~2e-3 extra rel err).
    yt = nc.dram_tensor("yt", [D, C], BF16, kind="ExternalOutput")

    xt_r = xt[:].rearrange("(kd p) c -> p kd c", p=P)
    xt_r_out = yt[:].rearrange("(kd p) c -> p kd c", p=P)

    Silu = mybir.ActivationFunctionType.Silu
    Mult = mybir.AluOpType.mult

    with tile.TileContext(nc) as tc:
        with (
            tc.tile_pool(name="resident", bufs=1) as resident,
            tc.tile_pool(name="wstrips", bufs=3) as wstrips,
            tc.tile_pool(name="tmp", bufs=3) as tmp,
            tc.tile_pool(name="psum", bufs=2, space="PSUM") as psum,
        ):
            xt_sb = resident.tile([P, KD, C], BF16, tag="xt")
            ht = resident.tile([P, FG, C], BF16, tag="ht")
            y_acc = resident.tile([P, KD, C], F32, tag="yacc")

            # Warm the PE clock: HAM un-throttles (1.2 -> 2.4 GHz) only after
            # ~3.4us of sustained matmul activity, so burn the initial DMA
            # wait on dummy matmuls — real MMs then start at full clock.
            warm = resident.tile([P, 512], BF16, tag="warm")
            nc.vector.memset(warm[:], 0.0)
            pswarm = psum.tile([P, 512], F32, tag="pswarm", name="pswarm",
                               bufs=1)
            for _ in range(18):
                nc.tensor.matmul(pswarm, warm[:, :P], warm[:],
                                 start=True, stop=True)
            for _ in range(8):
                # fine-grained tail: keeps the PE busy until the startup x
                # lands without coarse MMs delaying the first real matmul
                nc.tensor.matmul(pswarm[:, :P], warm[:, :P], warm[:, :P],
                                 start=True, stop=True)

            # Startup: the SDMA engines round-robin between ACTIVE queue
            # rings at packet granularity, so concurrent rings DIVIDE
            # bandwidth — and Tile's per-engine scheduling is work-
            # conserving, so a trigger on an idle engine fires immediately
            # no matter where it was emitted. The ONLY way to get strict
            # service order is one ring, emission-ordered: strip0, run-0 x,
            # run-0's remaining strips, then the other runs' x (emitted after
            # run 0's phase-1 strips below), then later strips.
            # Each HWDGE trigger costs ~0.6us of issuing-engine time, so the
            # critical path uses ONE descriptor per logical block.
            # Ring order = [w1 half of strip0][run-0 x][w3 half]: the ps1
            # kd-sweep needs only the w1 half + x, so the first real matmul
            # fires one strip-half (~1us) earlier; the w3 half lands while
            # the ps1 sweep runs. (SWDGE for strip0 measured ~6us first-byte
            # latency — strictly worse; keep everything on the SP ring.)
            # x leads: its completion semaphore (~2.8us write-receipt after
            # the transfer) is the critical gate for the first real matmul,
            # so its transfer starts first; the strip halves land under it.
            pre_strips = {}
            w13s0 = wstrips.tile([P, 2, KD, P], BF16, tag="w13s",
                                 name="w13s0", bufs=WBUFS)
            c0 = caps[0]
            nc.sync.dma_start(xt_sb[:, :, :c0], xt_r[:, :, :c0])
            nc.sync.dma_start(w13s0[:, 0], w13_d[0][0][:, 0])
            nc.sync.dma_start(w13s0[:, 1], w13_d[0][0][:, 1])
            pre_strips[(0, 0)] = w13s0

            def p1_chunk(w13s, ftl, lo, cw):
                ps1 = psum.tile([P, 512], F32, tag="ps1", name="ps1")[:, :cw]
                ps3 = psum.tile([P, 512], F32, tag="ps3", name="ps3")[:, :cw]
                for kd in range(KD):
                    nc.tensor.matmul(
                        ps1, w13s[:, 0, kd, :], xt_sb[:, kd, lo:lo + cw],
                        start=(kd == 0), stop=(kd == KD - 1),
                    )
                for kd in range(KD):
                    nc.tensor.matmul(
                        ps3, w13s[:, 1, kd, :], xt_sb[:, kd, lo:lo + cw],
                        start=(kd == 0), stop=(kd == KD - 1),
                    )
                h1t = tmp.tile([P, 512], BF16, tag="h1t", name="h1t")[:, :cw]
                nc.scalar.activation(h1t, ps1, Silu)
                nc.vector.tensor_tensor(ht[:, ftl, lo:lo + cw], h1t, ps3, op=Mult)

            def get_strip(r, kf):
                if (r, kf) in pre_strips:
                    return pre_strips.pop((r, kf))
                s = wstrips.tile([P, 2, KD, P], BF16, tag="w13s", bufs=WBUFS)
                nc.sync.dma_start(s[:], w13_d[r][kf])
                return s

            for g in range(NG):
                # ---- phase 1: H^T for this f-group, run-major ----
                for r in range(R):
                    off = offs[r]
                    for ftl in range(FG):
                        w13s = get_strip(r, g * FG + ftl)
                        for (cc, cw) in rchunks[r]:
                            p1_chunk(w13s, ftl, off + cc, cw)
                    if g == 0 and r == 0:
                        # runs 1..R-1's x, behind run 0's strips on the SAME
                        # ring: hardware-FIFO ordering keeps the startup
                        # strip stream at full bandwidth; these still land
                        # ~25us before run 1's first matmul needs them.
                        for rr in range(1, R):
                            o = offs[rr]
                            nc.sync.dma_start(
                                xt_sb[:, :, o:o + caps[rr]],
                                xt_r[:, :, o:o + caps[rr]],
                            )
                # ---- phase 2: accumulate Y^T contribution of this group ----
                # Last group: the final sum goes to a bf16 buffer — xt_sb is
                # fully consumed by now, so reuse it (zero extra SBUF) — and
                # yt streams out per dt-row on the otherwise-idle ACT ring,
                # keeping the SP ring free for w2 strips. Rows go out as FULL
                # 4KB-per-partition descriptors: narrower pieces drain at
                # ~1/4 rate (per-partition run = the SDMA packet).
                def p2_run(w2s, g, dtp, j, r):
                    off = offs[r]
                    dt = 2 * dtp + j
                    for (cc, cw) in rchunks[r]:
                        lo = off + cc
                        psy = psum.tile(
                            [P, 512], F32, tag="psy", name="psy", bufs=3
                        )[:, :cw]
                        for ftl in range(FG):
                            nc.tensor.matmul(
                                psy, w2s[:, j, ftl, :],
                                ht[:, ftl, lo:lo + cw],
                                start=(ftl == 0), stop=(ftl == FG - 1),
                            )
                        if g == 0:
                            nc.vector.tensor_copy(
                                y_acc[:, dt, lo:lo + cw], psy)
                        elif g < NG - 1:
                            nc.vector.tensor_add(
                                y_acc[:, dt, lo:lo + cw],
                                y_acc[:, dt, lo:lo + cw], psy,
                            )
                        else:
                            nc.vector.tensor_add(
                                xt_sb[:, dt, lo:lo + cw],
                                y_acc[:, dt, lo:lo + cw], psy,
                            )

                for dtp in range(KD // 2):
                    if not (g == NG - 1 and dtp == KD // 2 - 1):
                        for r in range(R):
                            w2s = wstrips.tile([P, 2, FG, P], BF16, tag="w2s",
                                               bufs=WBUFS)
                            nc.sync.dma_start(w2s[:], w2_d[r][g, dtp])
                            for j in range(2):
                                p2_run(w2s, g, dtp, j, r)
                        if g == NG - 1:
                            for j in range(2):
                                dt = 2 * dtp + j
                                nc.scalar.dma_start(
                                    xt_r_out[:, dt, :], xt_sb[:, dt, :])
                    else:
                        # final dt-pair: j-OUTER so dt6's full row streams
                        # out while dt7 computes, and dt7's row (the only
                        # drain after the last add) is one fast descriptor.
                        # All R strips fit in the w2s pool simultaneously.
                        strips = []
                        for r in range(R):
                            w2s = wstrips.tile([P, 2, FG, P], BF16, tag="w2s",
                                               bufs=WBUFS)
                            nc.sync.dma_start(w2s[:], w2_d[r][g, dtp])
                            strips.append(w2s)
                        for j in range(2):
                            for r in range(R):
                                p2_run(strips[r], g, dtp, j, r)
                            dt = 2 * dtp + j
                            nc.scalar.dma_start(
                                xt_r_out[:, dt, :], xt_sb[:, dt, :])
    nc.finalize()
    return nc


_NC_CACHE: dict = {}
last_results = None


def _install_ntff_shim():
    """This container's antenv lacks axon_hooks; recreate the NTFF profile
    hook from trn_boot's ctypes wrapper so trace=True yields profiles."""
    import types
    try:
        import antenv.axon_hooks  # noqa: F401
        return
    except ImportError:
        pass
    try:
        from trn_agent_boot.trn_boot import _ntff_profile_via_ctypes
        hook = _ntff_profile_via_ctypes("/opt/axon/libaxon_pjrt.so")
        mod = types.ModuleType("antenv.axon_hooks")
        mod.get_axon_ntff_profile_hook = lambda: hook
        mod.set_axon_ntff_profile_hook = lambda h: None
        sys.modules["antenv.axon_hooks"] = mod
    except Exception:
        pass


def _get_nc(D, F, caps, FG):
    key = (D, F, tuple(caps), FG)
    if key not in _NC_CACHE:
        _NC_CACHE[key] = build_ffn_nc(D, F, tuple(caps), FG)
    return _NC_CACHE[key]


def _softmax(z):
    e = np.exp(z - z.max(-1, keepdims=True))
    return e / e.sum(-1, keepdims=True)


def _strip_w13(w1, w3, dtype):
    """(D, F) x2 -> (KF, P, 2, KD, P): strip kf holds the w1 and w3 columns
    interleaved as one contiguous 512KB block, laid out exactly as the SBUF
    tile (partition-major, then w1/w3, then kd, then column)."""
    D, F = w1.shape
    KD, KF = D // P, F // P
    a = w1.reshape(KD, P, KF, P).transpose(2, 1, 0, 3)
    b = w3.reshape(KD, P, KF, P).transpose(2, 1, 0, 3)
    return np.ascontiguousarray(np.stack([a, b], axis=2)).astype(dtype)


def _strip_w2(w, FG, dtype):
    """(F, D) -> (NG, KD/2, P, 2, FG, P): strip (g, dtp) holds the dt-pair
    (2*dtp, 2*dtp+1) as one contiguous block, 4KB per partition line."""
    F, D = w.shape
    KD, KF = D // P, F // P
    NG = KF // FG
    return np.ascontiguousarray(
        w.reshape(NG, FG, P, KD // 2, 2, P).transpose(0, 3, 2, 4, 1, 5)
    ).astype(dtype)


def kernel(x, gate_w, w1, w3, w2):
    x = np.asarray(x, dtype=np.float32)
    gate_w = np.asarray(gate_w, dtype=np.float32)
    w1 = np.asarray(w1, dtype=np.float32)
    w3 = np.asarray(w3, dtype=np.float32)
    w2 = np.asarray(w2, dtype=np.float32)

    B, T, D = x.shape
    E, _, F = w1.shape
    N = B * T
    xf = x.reshape(N, D)

    # ---- host gate: softmax + top-2 + renormalize (tiny; replicated) ----
    logits = xf @ gate_w                      # (N, E)
    probs = _softmax(logits)
    top2 = np.argpartition(-probs, TOP_K - 1, axis=-1)[:, :TOP_K]  # (N, 2)
    pw = np.take_along_axis(probs, top2, axis=-1)
    pw = pw / pw.sum(-1, keepdims=True)       # renormalized weights

    # ---- dispatch: gather tokens per expert ----
    tok_ids, tok_wts = [], []
    for e in range(E):
        mask = (top2 == e)
        any_row = mask.any(-1)
        rows = np.nonzero(any_row)[0]
        wts = pw[any_row, :][mask[any_row, :]]
        tok_ids.append(rows)
        tok_wts.append(wts.astype(np.float32))
    counts = np.array([len(r) for r in tok_ids])

    # ---- group experts into two load-balanced quads; run r capacity is the
    # max quarter size over the two quads so one SPMD program fits all cores.
    G = N_CORES // SPLIT                      # number of quads (2)
    order = np.argsort(-counts, kind="stable")
    quads = [order[i::G] for i in range(G)]   # interleaved: balances run caps
    R = len(quads[0])
    caps = [int(-(-max(counts[quads[q][r]] for q in range(G)) // SPLIT))
            for r in range(R)]
    C = sum(caps)
    offs = [sum(caps[:r]) for r in range(R)]

    bf16 = ml_dtypes.bfloat16
    FG = 8
    wq = [(_strip_w13(w1[e], w3[e], bf16),
           _strip_w2(w2[e], FG, bf16)) for e in range(E)]

    nc = _get_nc(D, F, caps, FG)

    in_maps = []
    core_runs = []   # per core: list of (rows, wts, off) per run
    for c in range(N_CORES):
        q, quarter = c // SPLIT, c % SPLIT
        xt_c = np.zeros((D, C), dtype=bf16)
        im = {"xt": xt_c}
        runs = []
        for r in range(R):
            e = int(quads[q][r])
            qs = -(-counts[e] // SPLIT)       # quarter size for this expert
            rows = tok_ids[e][quarter * qs: (quarter + 1) * qs]
            wts = tok_wts[e][quarter * qs: (quarter + 1) * qs]
            xt_c[:, offs[r]: offs[r] + len(rows)] = xf[rows].T.astype(bf16)
            im[f"w13_{r}"], im[f"w2_{r}"] = wq[e]
            runs.append((rows, wts, offs[r]))
        in_maps.append(im)
        core_runs.append(runs)

    trace = os.environ.get("MOE_TRACE", "0") == "1"
    if trace:
        _install_ntff_shim()
    res = run_bass_kernel_spmd(nc, in_maps, list(range(N_CORES)), trace=trace)
    global last_results
    last_results = res

    out = np.zeros((N, D), dtype=np.float32)
    for c in range(N_CORES):
        y = np.asarray(res.results[c]["yt"], dtype=np.float32).T  # (C, D)
        for rows, wts, off in core_runs[c]:
            out[rows] += wts[:, None] * y[off: off + len(rows)]
    return out.reshape(B, T, D)

